# revision 42
# baseline (speedup 1.0000x reference)
"""Trainium2 Bass kernel for nn_Block_2010044694563 (dense transformer block).

B=4, S=2048, D=768, H=12 heads of 64. 8 NeuronCores, no collectives:
core c handles batch c//2, query-half c%2. Each core receives its batch's
2048 tokens rolled so its 1024 query rows come first, computes LN1 + K/V
over all 2048 local tokens (the only redundant work), attention for its
1024 queries x 12 heads, then out-proj + FFN on its 1024 rows.

The schedule is built around the ACT-engine exp stream (the hard floor,
~214us of ACTIVATE):
 - query-group-outer attention (2 groups of 512 queries), head pairs
   inner; the next head pair's K/Q projections are emitted as thunks
   into the current pair's j-loop so the PE never starves the exp
   stream. Score matmuls for a head pair run concurrently on row
   groups 0:64 / 64:128 of the PE array.
 - softmax: the ones-column of V makes PSUM row 64 the denominator.
   The PV PSUM is evacuated immediately (unnormalized, frees the
   bank); the normalize chain - reciprocal, a K=1 broadcast matmul
   against selection rows, one in-place multiply of oT - is deferred
   into the next head pair's slack.
 - LN rstd = Exp(-0.5*Ln(var+eps)); with the ACT table chooser pinned
   to natural_log_exp_and_others there are no table switches during
   the exp stream; exact Gelu runs as one tail batch (1 load).
 - LN2 + out-projection + FFN1 of query group 0 are emitted as thunks
   under group 1's attention. LN2 folds rstd into the PE transpose by
   multiplying with diag(rstd).
 - weight loads pay ~100ns un-hidden LDWEIGHTS per matmul on this
   stack, so matmuls are paired under one weight load wherever the
   stationary operand repeats (V/out-proj/FFN2 n-halves, K/Q nb
   groups), and LN1's PSUM evacuation copies run on the idle GpSimd.
"""

import numpy as np
import ml_dtypes

B, S, D, H = 4, 2048, 768, 12
HS = D // H           # 64
P = 128
NT = S                # local tokens per core (whole batch)
NQ = S // 2           # query tokens per core
TCH = NT // P         # 16 token chunks
KC = D // P           # 6 feature chunks
HPAIR = H // 2        # 6 head pairs
EPS = 1e-5
NEG = -1e9
SCALE = float(D) ** -0.5
BF16 = ml_dtypes.bfloat16

_PROGRAM_CACHE = {}


def _build_program():
    import concourse.bass as bass
    import concourse.mybir as mybir
    import concourse.tile as tile
    from concourse import bacc
    from concourse.masks import make_identity
    from contextlib import ExitStack
    from collections import deque

    f32 = mybir.dt.float32
    bf16 = mybir.dt.bfloat16
    AF = mybir.ActivationFunctionType
    OP = mybir.AluOpType

    nc = bacc.Bacc(None, target_bir_lowering=False)

    # Pin the ACT table-set chooser: the greedy pass would alternate
    # exp_and_others / natural_log per Exp/Ln instruction (~2.7us per
    # switch). natural_log_exp_and_others holds every function we use
    # outside the tail; set IDs stay index-stable.
    from concourse.hw_specs import get_activation_tables

    tables = get_activation_tables(nc.m.arch)
    keep = {"natural_log_exp_and_others", "gelu_and_others"}
    for name in list(tables):
        if name not in keep:
            tables[name] = set()

    x_d = nc.dram_tensor("x_local", [NT, D], f32, kind="ExternalInput")
    mb_d = nc.dram_tensor("maskbias", [NT], f32, kind="ExternalInput")
    wq_d = nc.dram_tensor("wq", [D, D], bf16, kind="ExternalInput")
    wk_d = nc.dram_tensor("wk", [D, D], bf16, kind="ExternalInput")
    wv_d = nc.dram_tensor("wv", [D, D], bf16, kind="ExternalInput")
    wo_d = nc.dram_tensor("wo", [D, D], bf16, kind="ExternalInput")
    w1_d = nc.dram_tensor("w1", [D, D], bf16, kind="ExternalInput")
    w2_d = nc.dram_tensor("w2", [D, D], bf16, kind="ExternalInput")
    bq_d = nc.dram_tensor("bq", [D], f32, kind="ExternalInput")
    bk_d = nc.dram_tensor("bk", [D], f32, kind="ExternalInput")
    bo_d = nc.dram_tensor("bo2", [D], f32, kind="ExternalInput")
    b1_d = nc.dram_tensor("b1f", [D], f32, kind="ExternalInput")
    b2_d = nc.dram_tensor("b2f", [D], f32, kind="ExternalInput")
    out_d = nc.dram_tensor("out", [NQ, D], f32, kind="ExternalOutput")

    with tile.TileContext(nc) as tc, ExitStack() as ctx:
        const = ctx.enter_context(tc.tile_pool(name="const", bufs=1))
        glob = ctx.enter_context(tc.tile_pool(name="glob", bufs=1))
        rot = ctx.enter_context(tc.tile_pool(name="rot", bufs=1))
        wpool = ctx.enter_context(tc.tile_pool(name="wpool", bufs=1))

        f16 = mybir.dt.float16

        # ---- constants ----
        ident = const.tile([P, P], bf16)
        make_identity(nc, ident)
        # head-pair selection rows (partition 0): slice 0 -> out rows 0:64,
        # slice 1 -> out rows 64:128
        sel_bc = const.tile([1, 2, P], bf16)
        nc.vector.memset(sel_bc, 0.0)
        nc.vector.memset(sel_bc[:, 0, 0:HS], 1.0)
        nc.vector.memset(sel_bc[:, 1, HS:P], 1.0)
        mb_sb = const.tile([P, TCH], f32)
        nc.sync.dma_start(out=mb_sb, in_=mb_d[:].rearrange("(c p) -> p c", p=P))
        bq_sb = const.tile([P, KC], f32)
        nc.sync.dma_start(out=bq_sb, in_=bq_d[:].rearrange("(c p) -> p c", p=P))
        bk_sb = const.tile([P, KC], f32)
        nc.sync.dma_start(out=bk_sb, in_=bk_d[:].rearrange("(c p) -> p c", p=P))
        b1_sb = const.tile([P, KC], f32)
        nc.sync.dma_start(out=b1_sb, in_=b1_d[:].rearrange("(c p) -> p c", p=P))
        bo_b = const.tile([P, D], f32)
        _bo = bo_d[:]
        nc.gpsimd.dma_start(
            out=bo_b, in_=bass.AP(tensor=_bo.tensor, offset=_bo.offset, ap=[[0, P], _bo.ap[0]])
        )
        b2_b = const.tile([P, D], f32)
        _b2 = b2_d[:]
        nc.gpsimd.dma_start(
            out=b2_b, in_=bass.AP(tensor=_b2.tensor, offset=_b2.offset, ap=[[0, P], _b2.ap[0]])
        )

        # ---- persistent activations ----
        qT = glob.tile([P, HPAIR, NQ], bf16)
        kT = glob.tile([P, HPAIR, NT], bf16)
        vA = glob.tile([P, TCH, H, HS + 1], bf16)   # [V | 1] per (chunk, head)
        oT = glob.tile([P, HPAIR, NQ], bf16)        # attention out^T
        xq = glob.tile([P, NQ // P, D], f32)        # x + bo; becomes x2; then out
        nc.vector.memset(vA[:, :, :, HS : HS + 1], 1.0)

        x_r = x_d[:].rearrange("(c p) d -> c p d", p=P)

        hpool = tc.alloc_tile_pool(name="hpool", bufs=1)
        hT = hpool.tile([P, KC, NT], bf16)

        ps_a = tc.alloc_tile_pool(name="ps_a", bufs=1, space="PSUM")

        def ln1_tile(t):
            xt = rot.tile([P, D], f32, tag="xin", bufs=3, name=f"xt{t}")
            nc.sync.dma_start(out=xt, in_=x_r[t])
            scr = rot.tile([P, D], bf16, tag="scr", bufs=2, name=f"scr{t}")
            ssq = rot.tile([P, 1], f32, tag="ssq", bufs=4, name=f"ssq{t}")
            nc.scalar.activation(scr, xt, AF.Square, accum_out=ssq)
            msum = rot.tile([P, 1], f32, tag="msum", bufs=4, name=f"msum{t}")
            nc.vector.reduce_sum(out=msum, in_=xt, axis=mybir.AxisListType.X)
            mu = rot.tile([P, 1], f32, tag="mu", bufs=4, name=f"mu{t}")
            nc.vector.tensor_scalar_mul(out=mu, in0=msum, scalar1=1.0 / D)
            mu2 = rot.tile([P, 1], f32, tag="mu2", bufs=4, name=f"mu2{t}")
            nc.vector.tensor_tensor(mu2, mu, mu, OP.mult)
            ve = rot.tile([P, 1], f32, tag="ve", bufs=4, name=f"ve_{t}")
            nc.vector.tensor_scalar(
                out=ve, in0=ssq, scalar1=1.0 / D, scalar2=EPS,
                op0=OP.mult, op1=OP.add,
            )
            nc.vector.tensor_tensor(ve, ve, mu2, OP.subtract)
            rstd = rot.tile([P, 1], f32, tag="rstd", bufs=4, name=f"rstd{t}")
            nc.scalar.activation(rstd, ve, AF.Ln)
            nc.scalar.activation(rstd, rstd, AF.Exp, scale=-0.5)
            nmr = rot.tile([P, 1], f32, tag="nmr", bufs=4, name=f"nmr{t}")
            nc.vector.tensor_scalar(
                out=nmr, in0=mu, scalar1=rstd, scalar2=-1.0,
                op0=OP.mult, op1=OP.mult,
            )
            xn = rot.tile([P, D], bf16, tag="xn", bufs=3, name=f"xn{t}")
            nc.scalar.activation(xn, xt, AF.Identity, bias=nmr, scale=rstd)
            pt = ps_a.tile([P, KC, P], bf16, tag="tp", bufs=2, name=f"pt{t}")
            for f in range(KC):
                nc.tensor.transpose(pt[:, f], xn[:, f * P : (f + 1) * P], ident)
            nc.vector.tensor_copy(out=hT[:, :, t * P : (t + 1) * P], in_=pt)

        def vproj_tile(t, wv_sb):
            # one hT-chunk weight load serves both n-halves
            pss = [
                ps_a.tile([P, 384], f32, tag="vv", bufs=2, name=f"psv{t}_{n2}")
                for n2 in range(2)
            ]
            for kc in range(KC):
                for n2 in range(2):
                    nc.tensor.matmul(
                        pss[n2],
                        lhsT=hT[:, kc, t * P : (t + 1) * P],
                        rhs=wv_sb[:, kc, n2 * 384 : (n2 + 1) * 384],
                        start=(kc == 0), stop=(kc == KC - 1),
                    )
            for n2 in range(2):
                nc.vector.tensor_copy(
                    out=vA[:, t, n2 * 6 : (n2 + 1) * 6, 0:HS],
                    in_=pss[n2].rearrange("p (h d) -> p h d", h=6),
                )

        # ---- paired K/Q projection emitters (2 psum tiles per weight pass) --
        def kproj_pair(hp, nbp, pool, wk_sb):
            pss = [
                pool.tile([P, 512], f32, tag="qk", bufs=2, name=f"psk{hp}_{nbp}_{i}")
                for i in range(2)
            ]
            for kc in range(KC):
                for i in range(2):
                    nc.tensor.matmul(
                        pss[i],
                        lhsT=wk_sb[:, kc, hp * P : (hp + 1) * P],
                        rhs=hT[:, kc, (2 * nbp + i) * 512 : (2 * nbp + i + 1) * 512],
                        start=(kc == 0), stop=(kc == KC - 1),
                    )
            for i in range(2):
                nc.vector.tensor_scalar_add(
                    out=kT[:, hp, (2 * nbp + i) * 512 : (2 * nbp + i + 1) * 512],
                    in0=pss[i], scalar1=bk_sb[:, hp : hp + 1],
                )

        def qproj_pair(hp, pool, wq_sb):
            pss = [
                pool.tile([P, 512], f32, tag="qk", bufs=2, name=f"psq{hp}_{i}")
                for i in range(2)
            ]
            for kc in range(KC):
                for i in range(2):
                    nc.tensor.matmul(
                        pss[i],
                        lhsT=wq_sb[:, kc, hp * P : (hp + 1) * P],
                        rhs=hT[:, kc, i * 512 : (i + 1) * 512],
                        start=(kc == 0), stop=(kc == KC - 1),
                    )
            for i in range(2):
                nc.vector.tensor_scalar_add(
                    out=qT[:, hp, i * 512 : (i + 1) * 512],
                    in0=pss[i], scalar1=bq_sb[:, hp : hp + 1],
                )

        # ================= Phase A: LN1 + V proj + hp0 projections ===========
        with nc.named_scope("ln1"):
            wv_sb = wq_sb = wk_sb = None
            for t in range(TCH):
                ln1_tile(t)
                if t == 0:
                    wv_sb = wpool.tile([P, KC, D], bf16, tag="w", bufs=3, name="wv_sb")
                    nc.sync.dma_start(
                        out=wv_sb, in_=wv_d[:].rearrange("(c p) n -> p c n", p=P)
                    )
                elif t == 2:
                    wq_sb = wpool.tile([P, KC, D], bf16, tag="w", bufs=3, name="wq_sb")
                    nc.sync.dma_start(
                        out=wq_sb, in_=wq_d[:].rearrange("(c p) n -> p c n", p=P)
                    )
                elif t == 4:
                    wk_sb = wpool.tile([P, KC, D], bf16, tag="w", bufs=3, name="wk_sb")
                    nc.sync.dma_start(
                        out=wk_sb, in_=wk_d[:].rearrange("(c p) n -> p c n", p=P)
                    )
                if t >= 1:
                    vproj_tile(t - 1, wv_sb)
                if t == 8:
                    qproj_pair(0, ps_a, wq_sb)
                elif t == 9:
                    kproj_pair(0, 0, ps_a, wk_sb)
            vproj_tile(TCH - 1, wv_sb)
            kproj_pair(0, 1, ps_a, wk_sb)
            for t in range(NQ // P):
                nc.sync.dma_start(out=xq[:, t], in_=x_r[t])
                nc.vector.tensor_tensor(xq[:, t], xq[:, t], bo_b, OP.add)

        ps_a.release()

        # ================= Attention: qc outer, head-pair inner ===============
        def attn_body(qc, hp, ps_pool, work, deferred, pvtail):
            """One head pair's attention for query group qc. `work` thunks
            drain into the exp-bound j-loop's PE slack; `deferred` holds the
            previous pair's normalize closure (run in this pair's slack)."""
            qs = slice(qc * 512, (qc + 1) * 512)
            pv = ps_pool.tile([HS + 1, 2, 512], f32, tag="pv", bufs=1, name=f"pv{qc}_{hp}")
            for j in range(TCH):
                sc = ps_pool.tile([P, 2, 512], f32, tag="sc", bufs=2, name=f"sc{qc}_{hp}_{j}")
                js = slice(j * P, (j + 1) * P)
                nc.tensor.matmul(
                    sc[:, 0, :], lhsT=kT[0:HS, hp, js], rhs=qT[0:HS, hp, qs],
                    start=True, stop=True,
                )
                nc.tensor.matmul(
                    sc[:, 1, :], lhsT=kT[HS:P, hp, js], rhs=qT[HS:P, hp, qs],
                    start=True, stop=True,
                )
                ex = rot.tile([P, 2, 512], bf16, tag="ex", bufs=3, name=f"ex{qc}_{hp}_{j}")
                nc.scalar.activation(
                    ex, sc, AF.Exp, bias=mb_sb[:, j : j + 1], scale=SCALE,
                )
                for h in range(2):
                    nc.tensor.matmul(
                        pv[:, h, :],
                        lhsT=vA[:, j, 2 * hp + h, :],
                        rhs=ex[:, h, :],
                        start=(j == 0), stop=(j == TCH - 1),
                    )
                if j == 2 and deferred[0] is not None:
                    deferred[0]()
                    deferred[0] = None
                if j % 2 == 1 and work:
                    work.popleft()()
            # ---- immediate evacuation (frees the pv bank fast) ----
            nc.vector.tensor_copy(out=oT[0:HS, hp, qs], in_=pv[0:HS, 0, :])
            nc.vector.tensor_copy(out=oT[HS:P, hp, qs], in_=pv[0:HS, 1, :])
            dn = rot.tile([1, 2, 512], f32, tag="dn", bufs=2, name=f"dn{qc}_{hp}")
            nc.vector.tensor_copy(out=dn, in_=pv[HS : HS + 1, :, :])

            def normalize():
                # reciprocal of the denominators, broadcast across partitions
                # via two K=1 matmuls against the selection rows, then one
                # in-place multiply of the unnormalized oT slice.
                dnr = rot.tile([1, 2, 512], f32, tag="dnr", bufs=2, name=f"dnr{qc}_{hp}")
                nc.vector.reciprocal_approx_fast(out=dnr, in_=dn)
                dnh = rot.tile([1, 2, 512], bf16, tag="dnh", bufs=2, name=f"dnh{qc}_{hp}")
                nc.vector.tensor_copy(out=dnh, in_=dnr)
                bc = ps_pool.tile([P, 512], f32, tag="sc", bufs=2, name=f"bc{qc}_{hp}")
                nc.tensor.matmul(
                    bc, lhsT=sel_bc[:, 0, :], rhs=dnh[:, 0, :], start=True, stop=False
                )
                nc.tensor.matmul(
                    bc, lhsT=sel_bc[:, 1, :], rhs=dnh[:, 1, :], start=False, stop=True
                )
                rcp = rot.tile([P, 512], bf16, tag="rcp", bufs=2, name=f"rcp{qc}_{hp}")
                nc.vector.tensor_copy(out=rcp, in_=bc)
                nc.vector.tensor_tensor(
                    oT[:, hp, qs], oT[:, hp, qs], rcp, OP.mult
                )

            return normalize

        # ---- query group 0: interleave remaining K/Q projections ----
        ps_qc0 = tc.alloc_tile_pool(name="ps_qc0", bufs=1, space="PSUM")
        deferred = [None]
        pvtail = [None]
        with nc.named_scope("attn0"):
            for hp in range(HPAIR):
                work = deque()
                if hp + 1 < HPAIR:
                    work.append(lambda hp=hp: qproj_pair(hp + 1, ps_qc0, wq_sb))
                    for nbp in range(2):
                        work.append(
                            lambda hp=hp, nbp=nbp: kproj_pair(hp + 1, nbp, ps_qc0, wk_sb)
                        )
                deferred[0] = attn_body(0, hp, ps_qc0, work, deferred, pvtail)
                while work:
                    work.popleft()()
            deferred[0]()
            deferred[0] = None
        ps_qc0.release()
        hpool.release()

        # ---- phase C machinery (thunks under group 1's attention) ----
        cpool = tc.alloc_tile_pool(name="cpool", bufs=1)
        h2T = cpool.tile([P, KC, NQ], bf16)
        gS = cpool.tile([P, KC, NQ], bf16)
        gT = h2T  # gelu output reuses h2T (fully read by FFN1 first)
        ps_c = tc.alloc_tile_pool(name="ps_c", bufs=1, space="PSUM")
        cps = {"pool": ps_c, "bufs": 2}

        wo_sb = wpool.tile([P, KC, D], bf16, tag="w", bufs=3, name="wo_sb")
        nc.sync.dma_start(out=wo_sb, in_=wo_d[:].rearrange("(c p) n -> p c n", p=P))
        w1_sb = wpool.tile([P, KC, D], bf16, tag="w", bufs=3, name="w1_sb")
        nc.sync.dma_start(out=w1_sb, in_=w1_d[:].rearrange("(c p) n -> p c n", p=P))
        w2_sb = wpool.tile([P, KC, D], bf16, tag="w", bufs=3, name="w2_sb")
        nc.sync.dma_start(out=w2_sb, in_=w2_d[:].rearrange("(c p) n -> p c n", p=P))

        def proj_qm(qm):
            # out-projection + residual for token chunk qm (kc-outer pair)
            pss = [
                cps["pool"].tile([P, 384], f32, tag="pj", bufs=cps["bufs"], name=f"pso{qm}_{n2}")
                for n2 in range(2)
            ]
            for kc in range(KC):
                for n2 in range(2):
                    nc.tensor.matmul(
                        pss[n2],
                        lhsT=oT[:, kc, qm * P : (qm + 1) * P],
                        rhs=wo_sb[:, kc, n2 * 384 : (n2 + 1) * 384],
                        start=(kc == 0), stop=(kc == KC - 1),
                    )
            for n2 in range(2):
                ns = slice(n2 * 384, (n2 + 1) * 384)
                nc.vector.tensor_tensor(xq[:, qm, ns], pss[n2], xq[:, qm, ns], OP.add)

        def ln2_qm(qm):
            x2 = xq[:, qm]
            ssq = rot.tile([P, 1], f32, tag="ssq", bufs=4, name=f"ssq2_{qm}")
            scr2 = rot.tile([P, D], bf16, tag="scr", bufs=2, name=f"scr2_{qm}")
            nc.scalar.activation(scr2, x2, AF.Square, accum_out=ssq)
            msum = rot.tile([P, 1], f32, tag="msum", bufs=4, name=f"msum2_{qm}")
            nc.vector.reduce_sum(out=msum, in_=x2, axis=mybir.AxisListType.X)
            mu = rot.tile([P, 1], f32, tag="mu", bufs=4, name=f"mu2_{qm}")
            nc.vector.tensor_scalar_mul(out=mu, in0=msum, scalar1=1.0 / D)
            mu2 = rot.tile([P, 1], f32, tag="mu2", bufs=4, name=f"mu2sq_{qm}")
            nc.vector.tensor_tensor(mu2, mu, mu, OP.mult)
            ve = rot.tile([P, 1], f32, tag="ve", bufs=4, name=f"ve2_{qm}")
            nc.vector.tensor_scalar(
                out=ve, in0=ssq, scalar1=1.0 / D, scalar2=EPS,
                op0=OP.mult, op1=OP.add,
            )
            nc.vector.tensor_tensor(ve, ve, mu2, OP.subtract)
            rstd = rot.tile([P, 1], f32, tag="rstd", bufs=4, name=f"rstd2_{qm}")
            nc.scalar.activation(rstd, ve, AF.Ln)
            nc.scalar.activation(rstd, rstd, AF.Exp, scale=-0.5)
            zn = rot.tile([P, D], bf16, tag="xn", bufs=3, name=f"zn{qm}")
            nc.vector.tensor_scalar(
                out=zn, in0=x2, scalar1=mu, scalar2=0.0,
                op0=OP.subtract, op1=OP.add,
            )
            idd = rot.tile([P, P], bf16, tag="idd", bufs=2, name=f"idd{qm}")
            nc.vector.tensor_scalar_mul(out=idd, in0=ident, scalar1=rstd)
            # h2T chunk = diag(rstd) folded into the transpose matmul
            for g in range(2):
                pt = cps["pool"].tile([P, 3, P], f32, tag="pj", bufs=cps["bufs"], name=f"pt2_{qm}_{g}")
                for f in range(3):
                    c = g * 3 + f
                    nc.tensor.matmul(
                        pt[:, f], lhsT=zn[:, c * P : (c + 1) * P], rhs=idd,
                        start=True, stop=True,
                    )
                nc.vector.tensor_copy(
                    out=h2T[:, g * 3 : (g + 1) * 3, qm * P : (qm + 1) * P], in_=pt
                )
            nc.vector.tensor_tensor(x2, x2, b2_b, OP.add)

        def ffn1_m(qc, m):
            qs = slice(qc * 512, (qc + 1) * 512)
            ps = cps["pool"].tile([P, 512], f32, tag="pj", bufs=cps["bufs"], name=f"psf{qc}_{m}")
            for kc in range(KC):
                nc.tensor.matmul(
                    ps,
                    lhsT=w1_sb[:, kc, m * P : (m + 1) * P],
                    rhs=h2T[:, kc, qs],
                    start=(kc == 0), stop=(kc == KC - 1),
                )
            nc.vector.tensor_copy(out=gS[:, m, qs], in_=ps)

        def phase_c_thunks(qc, split=True):
            th = []
            if split:
                # interleave variant: proj(qm) immediately followed by ln2(qm)
                for qm in range(qc * 4, qc * 4 + 4):
                    th.append(lambda qm=qm: proj_qm(qm))
                    th.append(lambda qm=qm: ln2_qm(qm))
            else:
                # tail variant: all projs back-to-back, then ln2 chains pipeline
                for qm in range(qc * 4, qc * 4 + 4):
                    th.append(lambda qm=qm: proj_qm(qm))
                for qm in range(qc * 4, qc * 4 + 4):
                    th.append(lambda qm=qm: ln2_qm(qm))
            for m in range(KC):
                th.append(lambda m=m: ffn1_m(qc, m))
            return th

        # ---- query group 1 with phase C(0) interleaved ----
        ps_qc1 = tc.alloc_tile_pool(name="ps_qc1", bufs=1, space="PSUM")
        cwork = deque(phase_c_thunks(0))
        with nc.named_scope("attn1"):
            for hp in range(HPAIR):
                work = deque()
                n_th = (1, 2, 2, 3, 3, 3)[hp]
                for _ in range(n_th):
                    if cwork:
                        work.append(cwork.popleft())
                deferred[0] = attn_body(1, hp, ps_qc1, work, deferred, pvtail)
                while work:
                    work.popleft()()
            deferred[0]()
            while cwork:
                cwork.popleft()()
        ps_qc1.release()
        ps_tail = tc.alloc_tile_pool(name="ps_tail", bufs=1, space="PSUM")
        cps["pool"], cps["bufs"] = ps_tail, 3

        # ================= Tail: phase C(1), gelu, FFN2, stores ===============
        with nc.named_scope("tail"):
            for th in phase_c_thunks(1, split=False):
                th()
            for m in range(KC):
                nc.scalar.activation(
                    gT[:, m, :], gS[:, m, :], AF.Gelu,
                    bias=b1_sb[:, m : m + 1], scale=1.0,
                )
            out_r = out_d[:].rearrange("(c p) d -> c p d", p=P)
            for qm in range(NQ // P):
                pss = [
                    ps_tail.tile([P, 384], f32, tag="pj", bufs=3, name=f"psg{qm}_{n2}")
                    for n2 in range(2)
                ]
                for kc in range(KC):
                    for n2 in range(2):
                        nc.tensor.matmul(
                            pss[n2],
                            lhsT=gT[:, kc, qm * P : (qm + 1) * P],
                            rhs=w2_sb[:, kc, n2 * 384 : (n2 + 1) * 384],
                            start=(kc == 0), stop=(kc == KC - 1),
                        )
                for n2 in range(2):
                    ns = slice(n2 * 384, (n2 + 1) * 384)
                    nc.vector.tensor_tensor(xq[:, qm, ns], pss[n2], xq[:, qm, ns], OP.add)
                nc.sync.dma_start(out=out_r[qm], in_=xq[:, qm])
        ps_tail.release()
        ps_c.release()
        cpool.release()

    nc.finalize()
    return nc


def _prepare_in_maps(inputs):
    x = np.ascontiguousarray(np.asarray(inputs["x"], dtype=np.float32))
    mask = np.asarray(inputs["attention_mask"])
    ln1_g = np.asarray(inputs["ln1_g"], dtype=np.float64)
    ln1_b = np.asarray(inputs["ln1_b"], dtype=np.float64)
    ln2_g = np.asarray(inputs["ln2_g"], dtype=np.float64)
    ln2_b = np.asarray(inputs["ln2_b"], dtype=np.float64)
    Wq = np.asarray(inputs["Wq"], dtype=np.float64)
    Wk = np.asarray(inputs["Wk"], dtype=np.float64)
    Wv = np.asarray(inputs["Wv"], dtype=np.float64)
    Wo = np.asarray(inputs["Wo"], dtype=np.float64)
    W1 = np.asarray(inputs["W1"], dtype=np.float64)
    W2 = np.asarray(inputs["W2"], dtype=np.float64)
    bo = np.asarray(inputs["bo"], dtype=np.float64)
    b1 = np.asarray(inputs["b1"], dtype=np.float64)
    b2 = np.asarray(inputs["b2"], dtype=np.float64)

    # fold LN gains/biases into the projection weights
    wq_f = (ln1_g[:, None] * Wq).astype(BF16)
    wk_f = (ln1_g[:, None] * Wk).astype(BF16)
    wv_f = (ln1_g[:, None] * Wv).astype(BF16)
    bq = (ln1_b @ Wq).astype(np.float32)
    bk = (ln1_b @ Wk).astype(np.float32)
    bv = ln1_b @ Wv
    wo_f = Wo.astype(BF16)
    bo2 = (bo + bv @ Wo).astype(np.float32)  # V-bias adds uniformly post-softmax
    w1_f = (ln2_g[:, None] * W1).astype(BF16)
    b1f = (b1 + ln2_b @ W1).astype(np.float32)
    w2_f = W2.astype(BF16)
    b2f = b2.astype(np.float32)

    maskbias = np.where(mask == 0, np.float32(NEG), np.float32(0.0)).astype(np.float32)

    in_maps = []
    for c in range(8):
        b, half = divmod(c, 2)
        xb = np.roll(x[b], -half * NQ, axis=0)
        mbb = np.roll(maskbias[b], -half * NQ, axis=0)
        in_maps.append(
            {
                "x_local": np.ascontiguousarray(xb),
                "maskbias": np.ascontiguousarray(mbb),
                "wq": wq_f, "wk": wk_f, "wv": wv_f, "wo": wo_f,
                "w1": w1_f, "w2": w2_f,
                "bq": bq, "bk": bk, "bo2": bo2, "b1f": b1f, "b2f": b2f,
            }
        )
    return in_maps


def run_on_cores(inputs, **spmd_kwargs):
    """Build (cached), run on cores 0-7, return (full_output, BassKernelResults)."""
    from concourse.bass_utils import run_bass_kernel_spmd

    if "nc" not in _PROGRAM_CACHE:
        _PROGRAM_CACHE["nc"] = _build_program()
    nc = _PROGRAM_CACHE["nc"]
    in_maps = _prepare_in_maps(inputs)
    res = run_bass_kernel_spmd(nc, in_maps, core_ids=list(range(8)), **spmd_kwargs)
    out = np.empty((B, S, D), dtype=np.float32)
    for c in range(8):
        b, half = divmod(c, 2)
        out[b, half * NQ : (half + 1) * NQ] = res.results[c]["out"]
    return out, res


def kernel(**inputs):
    out, _ = run_on_cores(inputs)
    return out
